# revision 2
# baseline (speedup 1.0000x reference)
"""Distributed Trainium2 attention-head kernel (softmax over the QUERY axis).

Strategy (8 NeuronCores, SPMD, query-dim sharding):
  - Host pre-tiles + bf16 hi/lo-splits the transposed activations and weights
    (exact relayout of the same fp32 data; 1/sqrt(qk) folded into qw).
  - Each core projects its own slab: qT = qw8.T @ queries_c.T, kT likewise,
    v = values_c @ vw (bf16).
  - kT (hi/lo stacked on partitions) + v are exchanged all-to-all with
    XOR-relative remote SBUF DMAs (no ncfw collective).
  - scoresT[j, i] = kT_hl.T @ q_hl via 2-pass stacked bf16-pair matmuls
    (fp32-grade accuracy, bf16 throughput), fp32 PSUM.
  - Softmax over i (axis 0 of scores = free axis of scoresT): per-tile DVE
    reduce_max(negate) -> ACT exp (bias=-max) with fused accum row-sums.
  - Per-column local stats (-max, sum) exchanged all-to-all; each core
    combines (global max via TT-min tree with XOR-permuted APs, rescaled
    sums), computes beta_j = exp(m_loc - M)/S, folds it into v.
  - outT[64, i] = sum_j vt[j, :].T-style accumulation: matmul(lhsT=vt tile,
    rhs=P tile) accumulated over all 64 j tiles in PSUM.
  - Host transposes/concats the 8 outT shards.
"""

import numpy as np

C = 8  # cores (XOR exchange assumes exactly 8)
QK = 64
VD = 64


def build_nc(seq=8192, d=1024, alias_inputs=True, comm=3, warmup=True):
    import concourse.bacc as bacc
    import concourse.mybir as mybir

    f32 = mybir.dt.float32
    bf16 = mybir.dt.bfloat16
    AX = mybir.AxisListType.X
    ALU = mybir.AluOpType
    ACTF = mybir.ActivationFunctionType

    SQL = seq // C          # queries per core / keys per core
    NDT = d // 128          # d_model tiles
    NT = seq // 128         # total j tiles
    TPB = SQL // 128        # j sub-tiles per block (per source core)
    CS = min(512, SQL)      # matmul N chunk
    NCH = SQL // CS         # chunks per i-range
    KVS = SQL + TPB * VD    # kv slot stride (kT cols + v cols), bf16
    IN_W = NDT * SQL        # input activation width per tensor

    nc = bacc.Bacc(target_bir_lowering=False, debug=False)

    # ---- DRAM params (all pre-tiled host-side) ----
    def din(name, w):
        return nc.declare_dram_parameter(name, [128, w], bf16, isOutput=False)

    qhT_d, qlT_d = din("qhT", IN_W), din("qlT", IN_W)
    khT_d, klT_d = din("khT", IN_W), din("klT", IN_W)
    vT_d = din("vT", IN_W)
    wqh_d, wql_d = din("wqh", NDT * QK), din("wql", NDT * QK)
    wkh_d, wkl_d = din("wkh", NDT * QK), din("wkl", NDT * QK)
    wv_d = din("wv", NDT * VD)
    out_d = nc.declare_dram_parameter("out", [VD, SQL], f32, isOutput=True)

    from contextlib import ExitStack

    with ExitStack() as ctx:
        block = ctx.enter_context(nc.Block())
        sem = lambda n: ctx.enter_context(nc.semaphore(n))
        sb = lambda n, shape, dt: ctx.enter_context(nc.sbuf_tensor(n, shape, dt))
        ps = lambda n, shape: ctx.enter_context(nc.psum_tensor(n, shape, f32))

        s_in_k = sem("s_in_k")
        s_in_k2 = sem("s_in_k2")
        s_in_q = sem("s_in_q")
        s_in_q2 = sem("s_in_q2")
        s_in_v = sem("s_in_v")
        s_in_out = sem("s_in_out")
        s_kproj = sem("s_kproj")
        s_qproj = sem("s_qproj")
        s_vproj = sem("s_vproj")
        s_ksplit = sem("s_ksplit")
        s_qsplit = sem("s_qsplit")
        s_vsplit = sem("s_vsplit")
        s_kldma = sem("s_kldma")
        s_qdma = sem("s_qdma")
        s_prep = sem("s_prep")
        s_krem = sem("s_krem")
        s_vrem2 = sem("s_vrem2")
        kvsems = [None] + [sem(f"s_kv{d}") for d in range(1, 8)]
        s_vrem = sem("s_vrem")
        s_srem = sem("s_srem")
        s_scpy = sem("s_scpy")
        s_rdloc = sem("s_rdloc")
        s_scores = sem("s_scores")
        s_max = sem("s_max")
        s_exp = sem("s_exp")
        s_dsub = sem("s_dsub")
        s_eexp = sem("s_eexp")
        s_vt = sem("s_vt")
        s_attn = sem("s_attn")
        s_outcp = sem("s_outcp")

        p_sb = sb("p_arena", [128, NT * SQL], bf16)
        kv_recv = sb("kv_recv", [128, C * KVS], bf16)
        qA = sb("qA", [128, SQL], bf16)
        qB = sb("qB", [128, SQL], bf16)
        kh2_tmp = sb("kh2_tmp", [128, SQL], bf16)
        wqh = sb("wqh_s", [128, NDT * QK], bf16)
        wql = sb("wql_s", [128, NDT * QK], bf16)
        wkh = sb("wkh_s", [128, NDT * QK], bf16)
        wkl = sb("wkl_s", [128, NDT * QK], bf16)
        wv = sb("wv_s", [128, NDT * VD], bf16)
        stats_mS = sb("stats_mS", [128, 2 * NT], f32)
        stats_recv = sb("stats_recv", [128, 7 * 2 * NT], f32)
        stats_send = sb("stats_send", [128, 7 * 2 * NT], f32)
        negM = sb("negM", [128, NT], f32)
        tmaxA = sb("tmaxA", [128, NT], f32)
        tmaxB = sb("tmaxB", [128, NT], f32)
        d_all = sb("d_all", [128, 8 * NT], f32)
        e_all = sb("e_all", [128, 8 * NT], f32)
        sw_all = sb("sw_all", [128, 8 * NT], f32)
        sg = sb("sg", [128, NT], f32)
        rS = sb("rS", [128, NT], f32)
        beta = sb("beta", [128, NT], f32)
        vt_sb = sb("vt_sb", [128, NT * VD], bf16)
        out_sb = sb("out_sb", [64, NCH * CS], f32)

        ps_A = ps("ps_A", [128, SQL])
        ps_B = ps("ps_B", [128, SQL])
        ps_C = ps("ps_C", [128, SQL])
        ps_o = ps("ps_o", [64, SQL])
        ps_v = ps_C  # v-projection uses ps_C[:, 0:TPB*VD] in phase 1
        # input-activation views (aliased into the P arena at full size)
        if alias_inputs:
            assert NT * SQL >= 5 * IN_W
            qhT = p_sb[:, 0 * IN_W : 1 * IN_W]
            qlT = p_sb[:, 1 * IN_W : 2 * IN_W]
            khT = p_sb[:, 2 * IN_W : 3 * IN_W]
            klT = p_sb[:, 3 * IN_W : 4 * IN_W]
            vT = p_sb[:, 4 * IN_W : 5 * IN_W]
            in_sb = {"qhT": qhT, "qlT": qlT, "khT": khT, "klT": klT, "vT": vT}
            extra = []
        else:
            extra = [
                sb(n, [128, IN_W], bf16)
                for n in ("qhT_s", "qlT_s", "khT_s", "klT_s", "vT_s")
            ]
            qhT, qlT, khT, klT, vT = (t[:, :] for t in extra)
            in_sb = {"qhT": qhT, "qlT": qlT, "khT": khT, "klT": klT, "vT": vT}

        # stats layout: col = block*2*TPB + {0..TPB-1: negm, TPB..2TPB-1: S}
        def col_m(tt):
            return (tt // TPB) * 2 * TPB + tt % TPB

        def col_S(tt):
            return col_m(tt) + TPB

        def half_view(t2d, base, part):
            """[p, 8, TPB] view of the negm ('m') or S ('S') half at base."""
            v = t2d[:, base : base + 2 * NT].rearrange(
                "p (b two t) -> p b two t", b=8, two=2, t=TPB
            )
            return v[:, :, 0 if part == "m" else 1]

        def comp_view(t2d, base):
            """[p, 8, TPB] view of a compact [128, NT] region at base."""
            return t2d[:, base : base + NT].rearrange(
                "p (b t) -> p b t", b=8, t=TPB
            )

        def xflip_src(t2d, base, bit):
            """XOR-by-single-bit view of a [128, 2*NT] stats region."""
            n = 2 * TPB  # block inner width
            if bit == 1:
                v = t2d[:, base : base + 2 * NT].rearrange(
                    "p (hb b0 t) -> p hb b0 t", hb=4, b0=2, t=n
                )
                return v[:, :, ::-1, :]
            if bit == 2:
                v = t2d[:, base : base + 2 * NT].rearrange(
                    "p (b2 b1 t) -> p b2 b1 t", b2=2, b1=2, t=2 * n
                )
                return v[:, :, ::-1, :]
            v = t2d[:, base : base + 2 * NT].rearrange(
                "p (b2 t) -> p b2 t", b2=2, t=4 * n
            )
            return v[:, ::-1, :]

        # ---------------- SYNC: input DMAs + output ----------------
        @block.sync
        def _(s):
            def dma(dst, src, sem_):
                s.dma_start(out=dst, in_=src[:, :]).then_inc(sem_, 16)

            H = IN_W // 2
            dma(wkh[:, :], wkh_d, s_in_k)
            dma(wkl[:, :], wkl_d, s_in_k)
            s.dma_start(out=khT[:, 0:H], in_=khT_d[:, 0:H]).then_inc(s_in_k, 16)
            s.dma_start(out=klT[:, 0:H], in_=klT_d[:, 0:H]).then_inc(s_in_k, 16)
            dma(wv[:, :], wv_d, s_in_v)
            dma(vT, vT_d, s_in_v)
            s.dma_start(out=khT[:, H:IN_W], in_=khT_d[:, H:IN_W]).then_inc(s_in_k2, 16)
            s.dma_start(out=klT[:, H:IN_W], in_=klT_d[:, H:IN_W]).then_inc(s_in_k2, 16)
            dma(wqh[:, :], wqh_d, s_in_q)
            dma(wql[:, :], wql_d, s_in_q)
            s.dma_start(out=qhT[:, 0:H], in_=qhT_d[:, 0:H]).then_inc(s_in_q, 16)
            s.dma_start(out=qlT[:, 0:H], in_=qlT_d[:, 0:H]).then_inc(s_in_q, 16)
            s.dma_start(out=qhT[:, H:IN_W], in_=qhT_d[:, H:IN_W]).then_inc(s_in_q2, 16)
            s.dma_start(out=qlT[:, H:IN_W], in_=qlT_d[:, H:IN_W]).then_inc(s_in_q2, 16)

            s.wait_ge(s_outcp, 1)
            s.dma_start(out=out_d[:, :], in_=out_sb[:, :]).then_inc(s_in_out, 16)
            s.wait_ge(s_in_out, 16)

        # ---------------- TENSOR ----------------
        @block.tensor
        def _(t):
            # HAM warm-up: junk matmuls while input DMAs stream so the k
            # projection runs at 2.4 GHz from its first instruction. Values
            # are garbage; k-proj's start=True clears the bank.
            for w in range(40 if warmup else 0):
                t.matmul(
                    ps_A[0:64, 0:CS],
                    p_sb[:, 0:64],
                    p_sb[:, 64 : 64 + CS],
                    start=(w == 0),
                    stop=False,
                )
            # k projection -> ps_A[0:64, :]; 3-term bf16 pair; dd-outer so
            # the first half of khT/klT suffices to start
            t.wait_ge(s_in_k, 64)
            for pos in (0, 64):
                for dd in range(NDT):
                    if dd == NDT // 2:
                        t.wait_ge(s_in_k2, 32)
                    for n in range(NCH):
                        for (W, X) in ((wkh, khT), (wkh, klT), (wkl, khT)):
                            mm = t.matmul(
                                ps_A[pos : pos + 64, n * CS : (n + 1) * CS],
                                W[:, dd * QK : (dd + 1) * QK],
                                X[:, dd * SQL + n * CS : dd * SQL + (n + 1) * CS],
                                start=(dd == 0 and W is wkh and X is khT),
                                stop=(dd == NDT - 1 and W is wkl),
                                tile_position=(0, pos),
                            )
            mm.then_inc(s_kproj, 1)

            # v projection -> ps_v [128, TPB*VD]
            t.wait_ge(s_in_v, 32)
            for tau in range(TPB):
                for dd in range(NDT):
                    mm = t.matmul(
                        ps_v[:, tau * VD : (tau + 1) * VD],
                        vT[:, dd * SQL + tau * 128 : dd * SQL + tau * 128 + 128],
                        wv[:, dd * VD : (dd + 1) * VD],
                        start=(dd == 0),
                        stop=(dd == NDT - 1),
                    )
            mm.then_inc(s_vproj, 1)

            # q projection -> ps_B[0:64, :]
            t.wait_ge(s_in_q, 64)
            for pos in (0, 64):
                for dd in range(NDT):
                    if dd == NDT // 2:
                        t.wait_ge(s_in_q2, 32)
                    for n in range(NCH):
                        for (W, X) in ((wqh, qhT), (wqh, qlT), (wql, qhT)):
                            mm = t.matmul(
                                ps_B[pos : pos + 64, n * CS : (n + 1) * CS],
                                W[:, dd * QK : (dd + 1) * QK],
                                X[:, dd * SQL + n * CS : dd * SQL + (n + 1) * CS],
                                start=(dd == 0 and W is wqh and X is qhT),
                                stop=(dd == NDT - 1 and W is wql),
                                tile_position=(0, pos),
                            )
            mm.then_inc(s_qproj, 1)

            # scores: tile tt (block delta, sub-tile tau) -> psum A/B alternating
            t.wait_ge(s_qsplit, 4)
            t.wait_ge(s_ksplit, 3)  # slot-0 kT complete
            t.wait_ge(s_vsplit, 1)  # ps_C free (v-proj results copied out)
            for tt in range(NT):
                delta, tau = divmod(tt, TPB)
                if tau == 0 and delta >= 1 and C > 1:
                    t.wait_ge(kvsems[delta], 2)  # this block's kv landed
                if tt >= 3:
                    t.wait_ge(s_exp, tt - 2)  # psum buffer free
                P = (ps_A, ps_B, ps_C)[tt % 3]
                lhs = kv_recv[:, delta * KVS + tau * 128 : delta * KVS + tau * 128 + 128]
                for n in range(NCH):
                    t.matmul(
                        P[:, n * CS : (n + 1) * CS],
                        lhs,
                        qA[:, n * CS : (n + 1) * CS],
                        start=True,
                        stop=False,
                    )
                    mm = t.matmul(
                        P[:, n * CS : (n + 1) * CS],
                        lhs,
                        qB[:, n * CS : (n + 1) * CS],
                        start=False,
                        stop=True,
                    )
                mm.then_inc(s_scores, 1)

            # attn@v: accumulate outT over all tiles
            for tt in range(NT):
                t.wait_ge(s_vt, tt + 1)
                for n in range(NCH):
                    mm = t.matmul(
                        ps_o[:, n * CS : (n + 1) * CS],
                        vt_sb[:, tt * VD : (tt + 1) * VD],
                        p_sb[:, tt * SQL + n * CS : tt * SQL + (n + 1) * CS],
                        start=(tt == 0),
                        stop=(tt == NT - 1),
                    )
            mm.then_inc(s_attn, 1)

        # ---------------- VECTOR ----------------
        @block.vector
        def _(v):
            # k split (kT duplicated on both psum halves; no partition moves)
            v.wait_ge(s_kproj, 1)
            v.tensor_copy(kv_recv[0:64, 0:SQL], ps_A[0:64, :]).then_inc(s_ksplit, 1)
            v.tensor_copy(kh2_tmp[64:128, :], ps_A[64:128, :]).then_inc(s_ksplit, 1)
            v.wait_ge(s_ksplit, 2)
            v.tensor_tensor(
                kv_recv[64:128, 0:SQL], ps_A[64:128, :], kh2_tmp[64:128, :],
                op=ALU.subtract,
            ).then_inc(s_ksplit, 1)
            # q split
            v.wait_ge(s_qproj, 1)
            v.tensor_copy(qA[0:64, :], ps_B[0:64, :]).then_inc(s_qsplit, 1)
            v.tensor_copy(qB[64:128, :], ps_B[64:128, :]).then_inc(s_qsplit, 1)
            v.wait_ge(s_qsplit, 2)
            v.tensor_tensor(
                qB[0:64, :], ps_B[0:64, :], qA[0:64, :], op=ALU.subtract
            ).then_inc(s_qsplit, 1)
            v.tensor_tensor(
                qA[64:128, :], ps_B[64:128, :], qB[64:128, :], op=ALU.subtract
            ).then_inc(s_qsplit, 1)
            # v split -> kv slot0 v part
            v.wait_ge(s_vproj, 1)
            v.tensor_copy(
                kv_recv[:, SQL : SQL + TPB * VD], ps_v[:, 0 : TPB * VD]
            ).then_inc(s_vsplit, 1)

            # per-tile negated max
            for tt in range(NT):
                v.wait_ge(s_scores, tt + 1)
                P = (ps_A, ps_B, ps_C)[tt % 3]
                v.reduce_max(
                    stats_mS[:, col_m(tt) : col_m(tt) + 1], P[:, :], axis=AX,
                    negate=True,
                ).then_inc(s_max, 1)

            # ---- permuted send copies (XOR chain), then stats combine ----
            v.wait_ge(s_exp, NT)
            chain = [0]

            def step(inst):
                inst.then_inc(s_dsub, 1)
                chain[0] += 1

            def cwait():
                v.wait_ge(s_dsub, chain[0])

            SS = 2 * NT  # send/recv slot stride

            def xcopy(dst_slot, src_t2d, src_base, bit):
                sv = xflip_src(src_t2d, src_base, bit)
                counts = [dim[1] for dim in sv.ap[1:]]
                names = "abcd"[: len(counts)]
                dst = stats_send[:, dst_slot * SS : (dst_slot + 1) * SS].rearrange(
                    f"p ({' '.join(names)}) -> p {' '.join(names)}",
                    **dict(zip(names, counts)),
                )
                step(v.tensor_copy(dst, sv))

            # P_d = XOR-d of stats_mS; chain: 1,2,3=1(2),4,5=1(4),6=2(4),7=1(6)
            xcopy(0, stats_mS, 0, 1)            # slot for delta=1
            xcopy(1, stats_mS, 0, 2)            # delta=2
            cwait()
            xcopy(2, stats_send, 1 * SS, 1)     # delta=3 = X1(P2)
            xcopy(3, stats_mS, 0, 4)            # delta=4
            cwait()
            xcopy(4, stats_send, 3 * SS, 1)     # delta=5 = X1(P4)
            xcopy(5, stats_send, 3 * SS, 2)     # delta=6 = X2(P4)
            cwait()
            xcopy(6, stats_send, 5 * SS, 1)     # delta=7 = X1(P6)
            cwait()
            v.sem_inc(s_scpy, 1)

            if C > 1:
                v.wait_ge(s_srem, 14)

            def slot_m(delta):
                if delta == 0:
                    return half_view(stats_mS, 0, "m")
                return half_view(stats_recv, (delta - 1) * SS, "m")

            def slot_S(delta):
                if delta == 0:
                    return half_view(stats_mS, 0, "S")
                return half_view(stats_recv, (delta - 1) * SS, "S")

            tA = comp_view(tmaxA, 0)
            tB = comp_view(tmaxB, 0)
            nM = comp_view(negM, 0)
            step(v.tensor_tensor(tA, slot_m(0), slot_m(1), op=ALU.min))
            step(v.tensor_tensor(tB, slot_m(2), slot_m(3), op=ALU.min))
            cwait()
            step(v.tensor_tensor(tA, tA, tB, op=ALU.min))
            cwait()
            step(v.tensor_tensor(tB, slot_m(4), slot_m(5), op=ALU.min))
            cwait()
            step(v.tensor_tensor(tA, tA, tB, op=ALU.min))
            cwait()
            step(v.tensor_tensor(tB, slot_m(6), slot_m(7), op=ALU.min))
            cwait()
            step(v.tensor_tensor(nM, tA, tB, op=ALU.min))
            cwait()
            # d_slot = negM - negm_slot  (compact, slot-major)
            for delta in range(8):
                step(
                    v.tensor_tensor(
                        comp_view(d_all, delta * NT), nM, slot_m(delta),
                        op=ALU.subtract,
                    )
                )
            nc._dsub_target = chain[0]  # ACT waits this
            # ACT computes e_all = exp(d_all); then weighted sums
            v.wait_ge(s_eexp, 1)
            for delta in range(8):
                step(
                    v.tensor_tensor(
                        comp_view(sw_all, delta * NT),
                        comp_view(e_all, delta * NT),
                        slot_S(delta),
                        op=ALU.mult,
                    )
                )
            cwait()
            # sum tree over 8 slots
            step(
                v.tensor_tensor(
                    sw_all[:, 0 : 4 * NT], sw_all[:, 0 : 4 * NT],
                    sw_all[:, 4 * NT : 8 * NT], op=ALU.add,
                )
            )
            cwait()
            step(
                v.tensor_tensor(
                    sw_all[:, 0 : 2 * NT], sw_all[:, 0 : 2 * NT],
                    sw_all[:, 2 * NT : 4 * NT], op=ALU.add,
                )
            )
            cwait()
            step(
                v.tensor_tensor(
                    sg[:, :], sw_all[:, 0:NT], sw_all[:, NT : 2 * NT], op=ALU.add
                )
            )
            cwait()
            step(v.reciprocal(rS[:, :], sg[:, :]))
            cwait()
            step(v.tensor_tensor(beta[:, :], e_all[:, 0:NT], rS[:, :], op=ALU.mult))
            cwait()

            # vt tiles: vt = v * beta (per-partition scalar), bf16
            if C > 1:
                v.wait_ge(s_vrem2, 14)
            v.wait_ge(s_vsplit, 1)
            for tt in range(NT):
                delta, tau = divmod(tt, TPB)
                v.tensor_scalar_mul(
                    vt_sb[:, tt * VD : (tt + 1) * VD],
                    kv_recv[
                        :,
                        delta * KVS + SQL + tau * VD : delta * KVS + SQL + (tau + 1) * VD,
                    ],
                    beta[:, tt : tt + 1],
                ).then_inc(s_vt, 1)

            # out copy
            v.wait_ge(s_attn, 1)
            v.tensor_copy(out_sb[:, :], ps_o[:, :]).then_inc(s_outcp, 1)

        # ---------------- SCALAR (ACT) ----------------
        @block.scalar
        def _(sc):
            for tt in range(NT):
                sc.wait_ge(s_max, tt + 1)
                P = (ps_A, ps_B, ps_C)[tt % 3]
                sc.activation(
                    p_sb[:, tt * SQL : (tt + 1) * SQL],
                    P[:, :],
                    ACTF.Exp,
                    bias=stats_mS[:, col_m(tt) : col_m(tt) + 1],
                    scale=1.0,
                    accum_out=stats_mS[:, col_S(tt) : col_S(tt) + 1],
                ).then_inc(s_exp, 1)
            # combine exp
            sc.wait_ge(s_dsub, nc._dsub_target)
            sc.activation(
                e_all[:, :], d_all[:, :], ACTF.Exp, scale=1.0
            ).then_inc(s_eexp, 1)

        # ---------------- GPSIMD: remote exchange (desc-gen up front) ----
        @block.gpsimd
        def _(g):
            if C > 1:
                def bcast(out_ap, in_ap, rsem, delta):
                    rd = [None] * 8
                    rd[delta] = (0, delta)
                    g.remote_dma_broadcast(
                        out_ap, in_ap, rsem, s_rdloc, rdests=rd
                    ).then_inc(s_prep, 1)

                # k sends first (gate: kl dma only), then v, then stats
                for delta in range(1, 8):
                    bcast(
                        kv_recv[:, delta * KVS : delta * KVS + SQL],
                        kv_recv[:, 0:SQL],
                        kvsems[delta],
                        delta,
                    )
                for delta in range(1, 8):
                    bcast(
                        kv_recv[:, delta * KVS + SQL : (delta + 1) * KVS],
                        kv_recv[:, SQL : SQL + TPB * VD],
                        s_vrem2,
                        delta,
                    )
                g.wait_ge(s_ksplit, 3)
                g.wait_ge(s_prep, 7)
                g.trigger_dma(7)
                # stats preps after the k trigger (ring holds <= 14 untriggered)
                for delta in range(1, 8):
                    bcast(
                        stats_recv[:, (delta - 1) * 2 * NT : delta * 2 * NT],
                        stats_send[:, (delta - 1) * 2 * NT : delta * 2 * NT],
                        s_srem,
                        delta,
                    )
                g.wait_ge(s_vsplit, 1)
                g.wait_ge(s_prep, 14)
                g.trigger_dma(7)
                g.wait_ge(s_scpy, 1)
                g.wait_ge(s_prep, 21)
                g.trigger_dma(7)
                g.wait_ge(s_rdloc, 21 * 16)

    nc.finalize()
    return nc


# ------------------------- host side -------------------------

def _split_bf16(x):
    import ml_dtypes

    hi = x.astype(ml_dtypes.bfloat16)
    lo = (x - hi.astype(np.float32)).astype(ml_dtypes.bfloat16)
    return hi, lo


def _tile_cols(xT, sql):
    """[d, sql] -> [128, (d//128)*sql], col dd*sql+i = xT[dd*128+p, i]."""
    dd = xT.shape[0] // 128
    return np.ascontiguousarray(
        xT.reshape(dd, 128, sql).transpose(1, 0, 2).reshape(128, dd * sql)
    )


def run_spmd_staged(nc, in_maps, profile_dir=None):
    """run_bass_via_pjrt with inputs pre-staged on-device (blocks until all
    shards are resident) so the 8 cores launch aligned instead of staggered
    by per-device input-transfer time. Optionally wraps the execute in the
    axon NTFF profile hook (profile_dir)."""
    import jax
    import numpy as np_
    from jax.sharding import Mesh, PartitionSpec, NamedSharding
    from jax.experimental.shard_map import shard_map
    import concourse.mybir as mybir
    from concourse import bass2jax

    bass2jax.install_neuronx_cc_hook()
    n_cores = len(in_maps)

    partition_name = (
        nc.partition_id_tensor.name if nc.partition_id_tensor else None
    )
    in_names, out_names, out_avals, zero_outs = [], [], [], []
    for alloc in nc.m.functions[0].allocations:
        if not isinstance(alloc, mybir.MemoryLocationSet):
            continue
        name = alloc.memorylocations[0].name
        if alloc.kind == "ExternalInput":
            if name != partition_name:
                in_names.append(name)
        elif alloc.kind == "ExternalOutput":
            out_names.append(name)
            shape = tuple(alloc.tensor_shape)
            dtype = mybir.dt.np(alloc.dtype)
            out_avals.append(jax.core.ShapedArray(shape, dtype))
            zero_outs.append(np_.zeros(shape, dtype))
    n_params = len(in_names)
    n_outs = len(out_avals)
    all_names = in_names + out_names
    if partition_name is not None:
        all_names = all_names + [partition_name]

    def _body(*args):
        operands = list(args)
        if partition_name is not None:
            operands.append(bass2jax.partition_id_tensor())
        outs = bass2jax._bass_exec_p.bind(
            *operands,
            out_avals=tuple(out_avals),
            in_names=tuple(all_names),
            out_names=tuple(out_names),
            lowering_input_output_aliases=(),
            sim_require_finite=True,
            sim_require_nnan=True,
            nc=nc,
        )
        return tuple(outs)

    devices = jax.devices()[:n_cores]
    mesh = Mesh(np_.asarray(devices), ("core",))
    spec = NamedSharding(mesh, PartitionSpec("core"))
    sharded = jax.jit(
        shard_map(
            _body,
            mesh=mesh,
            in_specs=(PartitionSpec("core"),) * (n_params + n_outs),
            out_specs=(PartitionSpec("core"),) * n_outs,
            check_rep=False,
        ),
        donate_argnums=tuple(range(n_params, n_params + n_outs)),
        keep_unused=True,
    )
    concat_in = [
        np_.concatenate([np_.asarray(in_maps[c][nm]) for c in range(n_cores)], axis=0)
        for nm in in_names
    ]
    concat_zero = [
        np_.zeros((n_cores * z.shape[0], *z.shape[1:]), z.dtype) for z in zero_outs
    ]
    staged = [jax.device_put(a, spec) for a in concat_in + concat_zero]
    jax.block_until_ready(staged)

    if profile_dir is not None:
        hook = None
        try:
            from antenv.axon_hooks import get_axon_ntff_profile_hook

            hook = get_axon_ntff_profile_hook()
        except ImportError:
            pass
        if hook is None:
            from trn_agent_boot.trn_boot import _ntff_profile_via_ctypes

            hook = _ntff_profile_via_ctypes("/opt/axon/libaxon_pjrt.so")
        with hook(profile_dir, list(range(n_cores))):
            out_arrs = sharded(*staged)
            jax.block_until_ready(out_arrs)
    else:
        out_arrs = sharded(*staged)
    return [
        {
            nm: np_.asarray(out_arrs[i]).reshape(n_cores, *out_avals[i].shape)[c]
            for i, nm in enumerate(out_names)
        }
        for c in range(n_cores)
    ]


def kernel(queries, keys, values, query_weights, key_weights, value_weights):
    import sys

    for p in ("/opt/trn_rl_repo",):
        if p not in sys.path:
            sys.path.insert(0, p)
    from concourse.bass_utils import run_bass_kernel_spmd

    seq, d = queries.shape
    sql = seq // C
    qw8 = (query_weights / np.sqrt(np.float32(QK))).astype(np.float32)

    wqh, wql = _split_bf16(qw8)
    wkh, wkl = _split_bf16(key_weights.astype(np.float32))
    import ml_dtypes

    wv = value_weights.astype(ml_dtypes.bfloat16)
    w_tiled = {
        "wqh": _tile_cols(wqh.astype(np.float32), QK),
        "wql": _tile_cols(wql.astype(np.float32), QK),
        "wkh": _tile_cols(wkh.astype(np.float32), QK),
        "wkl": _tile_cols(wkl.astype(np.float32), QK),
        "wv": _tile_cols(wv.astype(np.float32), VD),
    }
    w_tiled = {k: v.astype(ml_dtypes.bfloat16) for k, v in w_tiled.items()}

    in_maps = []
    for c in range(C):
        sl = slice(c * sql, (c + 1) * sql)
        qT = np.ascontiguousarray(queries[sl].T).astype(np.float32)
        kT = np.ascontiguousarray(keys[sl].T).astype(np.float32)
        vT = np.ascontiguousarray(values[sl].T).astype(np.float32)
        qh, ql = _split_bf16(qT)
        kh, kl = _split_bf16(kT)
        m = {
            "qhT": _tile_cols(qh.astype(np.float32), sql).astype(ml_dtypes.bfloat16),
            "qlT": _tile_cols(ql.astype(np.float32), sql).astype(ml_dtypes.bfloat16),
            "khT": _tile_cols(kh.astype(np.float32), sql).astype(ml_dtypes.bfloat16),
            "klT": _tile_cols(kl.astype(np.float32), sql).astype(ml_dtypes.bfloat16),
            "vT": _tile_cols(vT, sql).astype(ml_dtypes.bfloat16),
        }
        m.update(w_tiled)
        in_maps.append(m)

    nc = build_nc(seq=seq, d=d, alias_inputs=True)
    results = run_spmd_staged(nc, in_maps)
    out = np.concatenate(
        [np.asarray(results[c]["out"], dtype=np.float32).T for c in range(C)],
        axis=0,
    )
    return out



# revision 22
# speedup vs baseline: 33.3431x; 33.3431x over previous
"""Distributed Trainium2 attention-head kernel (softmax over the QUERY axis).

Strategy (8 NeuronCores, SPMD, KEY-dim sharding, zero cross-core comm):
  The softmax normalizes over the query axis (axis 0 of scores). Sharding
  the KEY dim keeps every softmax column fully local to one core: core c
  holds keys/values rows [c*1024:(c+1)*1024] and the FULL queries. It
  computes scoresT[j, i] for its 1024 keys x all 8192 queries, local
  per-key softmax stats, and a full-shape partial output
  outT_c = (attn_slab)^T-contracted with v_slab. The HOST sums the 8
  partials. No device-to-device traffic -> immune to the multi-ms
  execution-start stagger across the 8 PJRT devices (which dominated the
  previous all-to-all design at ~11.7ms).

  Numerics (validated vs reference in fp64/numpy: rel err 2.9e-3):
  - Host pre-splits activations/weights into bf16 hi/lo pairs.
  - 3-term bf16 projections (wh*xh + wh*xl + wl*xh) in fp32 PSUM.
  - Projected q/k re-split hi/lo on device via PSUM bf16-rounding trick.
  - Scores: 2-pass stacked bf16 matmuls: kT_A=[kh;kl], kT_B=[kl;kh]
    against qS=[qh;ql] -> exact (kh+kl)(qh+ql) in fp32 PSUM.
  - Softmax over queries with per-512-chunk max; the exp(m_chunk - m_j)/S_j
    rescale folds into per-chunk vt tiles used as attn@v lhsT.
"""

import numpy as np

C = 8
SEQ = 8192
D = 1024
QK = 64
VD = 64


def build_nc(seq=SEQ, d=D, warmup=True, debug_taps=False):
    import concourse.bacc as bacc
    import concourse.mybir as mybir

    f32 = mybir.dt.float32
    bf16 = mybir.dt.bfloat16
    AX = mybir.AxisListType.X
    ALU = mybir.AluOpType
    ACTF = mybir.ActivationFunctionType

    NDT = d // 128            # 8 d_model tiles
    KSL = seq // C            # 1024 keys per core
    KT = KSL // 128           # 8 key tiles
    NSEC = 4                  # q-projection sections
    SECW = seq // NSEC        # 2048 query cols per section
    NCHK = seq // 512         # 16 score chunks per key tile
    NIDX = KT * NCHK          # 128 (j, c) chunk pairs
    QIN_W = NDT * seq         # 65536 q activation cols
    KIN_W = NDT * KSL         # 8192 k/v activation cols
    NPIECE = NSEC * NDT * 2   # 64 streamed q pieces

    nc = bacc.Bacc(target_bir_lowering=False, debug=False)

    def din(name, w):
        return nc.declare_dram_parameter(name, [128, w], bf16, isOutput=False)

    qhT_d, qlT_d = din("qhT", QIN_W), din("qlT", QIN_W)
    khT_d, klT_d = din("khT", KIN_W), din("klT", KIN_W)
    vT_d = din("vT", KIN_W)
    wqh_d, wql_d = din("wqh", NDT * QK), din("wql", NDT * QK)
    wkh_d, wkl_d = din("wkh", NDT * QK), din("wkl", NDT * QK)
    wv_d = din("wv", NDT * VD)
    out_d = nc.declare_dram_parameter("out", [VD, seq], f32, isOutput=True)
    dbg = {}
    if debug_taps:
        def dout(name, p, w, dt):
            dbg[name] = nc.declare_dram_parameter(name, [p, w], dt, isOutput=True)

        dout("d_qS", 128, seq, bf16)
        dout("d_kTA", 128, seq // C, bf16)
        dout("d_kTB", 128, seq // C, bf16)
        dout("d_vsb", 128, (seq // C // 128) * VD, bf16)
        dout("d_negm", 128, (seq // C // 128) * (seq // 512), f32)
        dout("d_S", 128, (seq // C // 128) * (seq // 512), f32)
        dout("d_emat", 128, (seq // C // 128) * (seq // 512), f32)
        dout("d_mt1", 128, (seq // C // 128) * 8, f32)
        dout("d_negM", 128, seq // C // 128, f32)
        dout("d_dmat", 128, (seq // C // 128) * (seq // 512), f32)
        dout("d_Sg", 128, seq // C // 128, f32)
        dout("d_gg", 128, (seq // C // 128) * (seq // 512), f32)
        dout("d_vt", 128, (seq // C // 128) * (seq // 512) * VD, bf16)
        dout("d_P", 128, (seq // C // 128) * seq, bf16)

    from contextlib import ExitStack

    with ExitStack() as ctx:
        block = ctx.enter_context(nc.Block())
        sem = lambda n: ctx.enter_context(nc.semaphore(n))
        sb = lambda n, shape, dt: ctx.enter_context(nc.sbuf_tensor(n, shape, dt))
        ps = lambda n, shape: ctx.enter_context(nc.psum_tensor(n, shape, f32))

        s_in_k = sem("s_in_k")
        s_in_v = sem("s_in_v")
        s_in_q = sem("s_in_q")
        # per-slot DMA-completion sems: a single counting sem is racy when
        # two DMAs are in flight (16 per-engine incs interleave across DMAs)
        s_qd = [sem(f"s_qd{i}") for i in range(3)]
        s_odp = [sem("s_od0"), sem("s_od1")]
        s_qcons = sem("s_qcons")
        s_qsp = sem("s_qsp")
        s_kproj = sem("s_kproj")
        s_ksp = sem("s_ksp")
        s_vproj = sem("s_vproj")
        s_vcp = sem("s_vcp")
        s_sc = sem("s_sc")
        s_mx = sem("s_mx")
        s_exp = sem("s_exp")
        s_dm = sem("s_dm")
        s_em = sem("s_em")
        s_mM = sem("s_mM")
        s_rs = sem("s_rs")
        s_gq = sem("s_gq")
        s_vch = sem("s_vch")
        s_vt = sem("s_vt")
        s_attn = sem("s_attn")
        s_ocp = sem("s_ocp")
        s_od = sem("s_od")

        # ---- SBUF ----
        p_sb = sb("p_arena", [128, KT * seq], bf16)     # P; aliases k/v acts
        khT = p_sb[:, 0 * KIN_W : 1 * KIN_W]
        klT = p_sb[:, 1 * KIN_W : 2 * KIN_W]
        vT = p_sb[:, 2 * KIN_W : 3 * KIN_W]
        qS = sb("qS", [128, seq], bf16)                 # [qh(0:64); ql(64:128)]
        qslot = sb("qslot", [128, 3 * SECW], bf16)      # 3 streaming slots
        tmpq = sb("tmpq", [128, SECW], bf16)            # split scratch (64:128)
        kT_A = sb("kT_A", [128, KSL], bf16)             # [kh; kl]
        kT_B = sb("kT_B", [128, KSL], bf16)             # [kl; kh]
        wqh = sb("wqh_s", [128, NDT * QK], bf16)
        wql = sb("wql_s", [128, NDT * QK], bf16)
        wkh = sb("wkh_s", [128, NDT * QK], bf16)
        wkl = sb("wkl_s", [128, NDT * QK], bf16)
        wv = sb("wv_s", [128, NDT * VD], bf16)
        v_sb = sb("v_sb", [128, KT * VD], bf16)         # projected v slab
        vt_all = sb("vt_all", [128, NIDX * VD], bf16)   # per-(j,c) vt tiles
        negm = sb("negm", [128, NIDX], f32)             # -max per (j,c)
        S_ = sb("S_", [128, NIDX], f32)                 # exp-sum per (j,c)
        mt1 = sb("mt1", [128, KT * 8], f32)
        mt2 = sb("mt2", [128, KT * 4], f32)
        mt3 = sb("mt3", [128, KT * 2], f32)
        negM = sb("negM", [128, KT], f32)               # -max per j
        dmat = sb("dmat", [128, NIDX], f32)             # m_j - m_jc
        emat = sb("emat", [128, NIDX], f32)             # exp(m_jc - m_j)
        Sw = sb("Sw", [128, NIDX], f32)
        Sg = sb("Sg", [128, KT], f32)
        rS = sb("rS", [128, KT], f32)
        gg = sb("gg", [128, NIDX], f32)                 # e/S fold factors
        out_sb = sb("out_sb", [64, 2 * 1024], f32)

        # ---- PSUM: two 4-bank halves, views per phase ----
        ps_qA = ps("ps_qA", [128, SECW])
        ps_qB = ps("ps_qB", [128, SECW])
        kps = ps_qA[:, 0:KSL]                 # k-projection [128, 1024]
        vps = ps_qA[:, KSL : KSL + KT * VD]   # v-projection [128, 512]
        sbank = [ps_qA[:, 0:512], ps_qA[:, 512:1024], ps_qA[:, 1024:1536]]
        # all at partition base 0: DVE copies to out_sb must stay lane-aligned
        atile = [
            ps_qB[0:64, 0:1024],
            ps_qB[0:64, 1024:2048],
            ps_qA[0:64, 0:1024],
            ps_qA[0:64, 1024:2048],
        ]

        def jv3(t2d, j, c):
            return t2d[:, 0 : j * c].rearrange("p (j c) -> p j c", j=j, c=c)

        # ---------------- SYNC: k/v/w input DMAs + output ----------------
        @block.sync
        def _(s):
            def dma(dst, src, sem_):
                s.dma_start(out=dst, in_=src).then_inc(sem_, 16)

            dma(wkh[:, :], wkh_d[:, :], s_in_k)
            dma(wkl[:, :], wkl_d[:, :], s_in_k)
            dma(khT, khT_d[:, :], s_in_k)
            dma(klT, klT_d[:, :], s_in_k)
            dma(wv[:, :], wv_d[:, :], s_in_v)
            dma(vT, vT_d[:, :], s_in_v)
            dma(wqh[:, :], wqh_d[:, :], s_in_q)
            dma(wql[:, :], wql_d[:, :], s_in_q)
            # output tiles
            for t in range(8):
                s.wait_ge(s_ocp, t + 1)
                s.dma_start(
                    out=out_d[:, t * 1024 : (t + 1) * 1024],
                    in_=out_sb[:, (t % 2) * 1024 : (t % 2 + 1) * 1024],
                ).then_inc(s_odp[t % 2], 16)
            s.wait_ge(s_odp[0], 4 * 16)
            s.wait_ge(s_odp[1], 4 * 16)
            if debug_taps:
                n_dbg = 0
                for name, src in (
                    ("d_qS", qS[:, :]),
                    ("d_kTA", kT_A[:, :]),
                    ("d_kTB", kT_B[:, :]),
                    ("d_vsb", v_sb[:, :]),
                    ("d_negm", negm[:, :]),
                    ("d_S", S_[:, :]),
                    ("d_emat", emat[:, :]),
                    ("d_mt1", mt1[:, :]),
                    ("d_negM", negM[:, :]),
                    ("d_dmat", dmat[:, :]),
                    ("d_Sg", Sg[:, :]),
                    ("d_gg", gg[:, :]),
                    ("d_vt", vt_all[:, :]),
                    ("d_P", p_sb[:, :]),
                ):
                    s.dma_start(out=dbg[name][:, :], in_=src).then_inc(s_odp[0], 16)
                    n_dbg += 1
                s.wait_ge(s_odp[0], (4 + n_dbg) * 16)

        # ---------------- SCALAR (ACT): q-piece DMAs + exp ----------------
        @block.scalar
        def _(sc):
            # q stream on the ACT HWDGE ring, gated behind k/v so those get
            # full bandwidth first.
            sc.wait_ge(s_in_k, 64)
            sc.wait_ge(s_in_v, 32)
            for p in range(NPIECE):
                sec, rem = divmod(p, NDT * 2)
                dd, hl = divmod(rem, 2)
                if p >= 3:
                    sc.wait_ge(s_qcons, p - 2)
                src_d = qhT_d if hl == 0 else qlT_d
                sc.dma_start(
                    out=qslot[:, (p % 3) * SECW : (p % 3 + 1) * SECW],
                    in_=src_d[:, dd * seq + sec * SECW : dd * seq + (sec + 1) * SECW],
                ).then_inc(s_qd[p % 3], 16)

            # exp per (j, c) chunk
            for idx in range(NIDX):
                j, c = divmod(idx, NCHK)
                sc.wait_ge(s_mx, idx + 1)
                sc.activation(
                    p_sb[:, j * seq + c * 512 : j * seq + (c + 1) * 512],
                    sbank[idx % 3],
                    ACTF.Exp,
                    bias=negm[:, idx : idx + 1],
                    scale=1.0,
                    accum_out=S_[:, idx : idx + 1],
                ).then_inc(s_exp, 1)
            # combine: emat = exp(-dmat)
            sc.wait_ge(s_dm, 1)
            sc.activation(
                emat[:, :], dmat[:, :], ACTF.Exp, scale=-1.0
            ).then_inc(s_em, 1)

        # ---------------- TENSOR ----------------
        @block.tensor
        def _(t):
            # HAM warm-up junk matmuls while k/v DMAs stream.
            for w in range(40 if warmup else 0):
                t.matmul(
                    kps[0:64, 0:512],
                    p_sb[:, 0:64],
                    p_sb[:, 64 : 64 + 512],
                    start=(w == 0),
                    stop=False,
                )

            # k projection -> kps, duplicated on both partition halves.
            t.wait_ge(s_in_k, 64)
            for pos in (0, 64):
                for dd in range(NDT):
                    for ti, (W, X) in enumerate(
                        ((wkh, khT), (wkh, klT), (wkl, khT))
                    ):
                        for n in range(2):
                            mm = t.matmul(
                                kps[pos : pos + 64, n * 512 : (n + 1) * 512],
                                W[:, dd * QK : (dd + 1) * QK],
                                X[:, dd * KSL + n * 512 : dd * KSL + (n + 1) * 512],
                                start=(dd == 0 and ti == 0),
                                stop=(dd == NDT - 1 and ti == 2),
                                tile_position=(0, pos),
                            )
            mm.then_inc(s_kproj, 1)

            # v projection -> vps
            t.wait_ge(s_in_v, 32)
            for tau in range(KT):
                for dd in range(NDT):
                    mm = t.matmul(
                        vps[:, tau * VD : (tau + 1) * VD],
                        vT[:, dd * KSL + tau * 128 : dd * KSL + tau * 128 + 128],
                        wv[:, dd * VD : (dd + 1) * VD],
                        start=(dd == 0),
                        stop=(dd == NDT - 1),
                    )
            mm.then_inc(s_vproj, 1)

            # q projection in 4 sections of 2048, ping-pong PSUM halves
            t.wait_ge(s_in_q, 32)
            t.wait_ge(s_ksp, 1)   # section 0 overwrites kps
            t.wait_ge(s_vcp, 1)   # ... and vps
            for sec in range(NSEC):
                if sec >= 2:
                    t.wait_ge(s_qsp, sec - 1)
                buf = (ps_qA, ps_qB)[sec % 2]
                pbase = sec * NDT * 2
                for dd in range(NDT):
                    pq = pbase + dd * 2
                    t.wait_ge(s_qd[pq % 3], 16 * (pq // 3 + 1))
                    qh_s = qslot[:, (pq % 3) * SECW : (pq % 3 + 1) * SECW]
                    ql_s = qslot[:, ((pq + 1) % 3) * SECW : ((pq + 1) % 3 + 1) * SECW]
                    for ti, (W, X) in enumerate(
                        ((wqh, qh_s), (wql, qh_s), (wqh, ql_s))
                    ):
                        if ti == 2:
                            t.wait_ge(s_qd[(pq + 1) % 3], 16 * ((pq + 1) // 3 + 1))
                        for n in range(4):
                            for pos in (0, 64):
                                mm = t.matmul(
                                    buf[pos : pos + 64, n * 512 : (n + 1) * 512],
                                    W[:, dd * QK : (dd + 1) * QK],
                                    X[:, n * 512 : (n + 1) * 512],
                                    start=(dd == 0 and ti == 0),
                                    stop=(dd == NDT - 1 and ti == 2),
                                    tile_position=(0, pos),
                                )
                        if ti >= 1:  # qh consumed after ti=1, ql after ti=2
                            mm.then_inc(s_qcons, 1)

            # scores: 2-pass stacked bf16, 3-bank rotation
            t.wait_ge(s_qsp, NSEC)
            for idx in range(NIDX):
                j, c = divmod(idx, NCHK)
                if idx >= 3:
                    t.wait_ge(s_exp, idx - 2)
                bank = sbank[idx % 3]
                t.matmul(
                    bank,
                    kT_A[:, j * 128 : (j + 1) * 128],
                    qS[:, c * 512 : (c + 1) * 512],
                    start=True,
                    stop=False,
                )
                mm = t.matmul(
                    bank,
                    kT_B[:, j * 128 : (j + 1) * 128],
                    qS[:, c * 512 : (c + 1) * 512],
                    start=False,
                    stop=True,
                )
                mm.then_inc(s_sc, 1)

            # attn@v: 8 output tiles of [64, 1024], accumulate over j
            t.wait_ge(s_vt, 1)
            for tt in range(8):
                if tt >= 4:
                    t.wait_ge(s_ocp, tt - 3)
                pt = atile[tt % 4]
                for half in range(2):
                    cc = tt * 2 + half
                    for j in range(KT):
                        mm = t.matmul(
                            pt[:, half * 512 : (half + 1) * 512],
                            vt_all[
                                :, (j * NCHK + cc) * VD : (j * NCHK + cc + 1) * VD
                            ],
                            p_sb[:, j * seq + cc * 512 : j * seq + (cc + 1) * 512],
                            start=(j == 0),
                            stop=(j == KT - 1),
                            tile_position=(0, 0),
                        )
                mm.then_inc(s_attn, 1)

        # ---------------- VECTOR ----------------
        @block.vector
        def _(v):
            # The DVE has no intra-engine RAW interlock: any op reading data
            # written by a recent DVE op must sit behind a completion fence
            # (then_inc at writeback + wait_ge). s_vch is the chain counter.
            vch = [0]

            def vfence(inst):
                inst.then_inc(s_vch, 1)
                vch[0] += 1
                v.wait_ge(s_vch, vch[0])

            # k hi/lo split, both stackings
            v.wait_ge(s_kproj, 1)
            v.tensor_copy(kT_A[0:64, :], kps[0:64, :])
            vfence(v.tensor_copy(kT_B[64:128, :], kps[64:128, :]))
            v.tensor_tensor(
                kT_A[64:128, :], kps[64:128, :], kT_B[64:128, :], op=ALU.subtract
            )
            v.tensor_tensor(
                kT_B[0:64, :], kps[0:64, :], kT_A[0:64, :], op=ALU.subtract
            ).then_inc(s_ksp, 1)

            # v copy
            v.wait_ge(s_vproj, 1)
            v.tensor_copy(v_sb[:, :], vps[:, :]).then_inc(s_vcp, 1)

            # q splits per section
            for sec in range(NSEC):
                v.wait_ge(s_qcons, 2 * NDT * (sec + 1))
                buf = (ps_qA, ps_qB)[sec % 2]
                v.tensor_copy(qS[0:64, sec * SECW : (sec + 1) * SECW], buf[0:64, :])
                vfence(v.tensor_copy(tmpq[64:128, :], buf[64:128, :]))
                v.tensor_tensor(
                    qS[64:128, sec * SECW : (sec + 1) * SECW],
                    buf[64:128, :],
                    tmpq[64:128, :],
                    op=ALU.subtract,
                ).then_inc(s_qsp, 1)

            # per-chunk negated max
            for idx in range(NIDX):
                v.wait_ge(s_sc, idx + 1)
                v.reduce_max(
                    negm[:, idx : idx + 1], sbank[idx % 3], axis=AX, negate=True
                ).then_inc(s_mx, 1)

            # combine stats
            v.wait_ge(s_exp, NIDX)
            nv = jv3(negm, KT, NCHK)
            vfence(
                v.tensor_tensor(
                    jv3(mt1, KT, 8), nv[:, :, 0:8], nv[:, :, 8:16], op=ALU.min
                )
            )
            m1 = jv3(mt1, KT, 8)
            vfence(
                v.tensor_tensor(
                    jv3(mt2, KT, 4), m1[:, :, 0:4], m1[:, :, 4:8], op=ALU.min
                )
            )
            m2 = jv3(mt2, KT, 4)
            vfence(
                v.tensor_tensor(
                    jv3(mt3, KT, 2), m2[:, :, 0:2], m2[:, :, 2:4], op=ALU.min
                )
            )
            m3 = jv3(mt3, KT, 2)
            vfence(
                v.tensor_tensor(
                    jv3(negM, KT, 1), m3[:, :, 0:1], m3[:, :, 1:2], op=ALU.min
                )
            )
            for j in range(KT):
                ins = v.tensor_scalar_sub(
                    dmat[:, j * NCHK : (j + 1) * NCHK],
                    negm[:, j * NCHK : (j + 1) * NCHK],
                    negM[:, j : j + 1],
                )
            ins.then_inc(s_dm, 1)
            v.wait_ge(s_em, 1)
            vfence(v.tensor_tensor(Sw[:, :], emat[:, :], S_[:, :], op=ALU.mult))
            for j in range(KT):
                ins = v.reduce_sum(
                    Sg[:, j : j + 1], Sw[:, j * NCHK : (j + 1) * NCHK], axis=AX
                )
            vfence(ins)
            v.reciprocal(rS[:, :], Sg[:, :]).then_inc(s_rs, 1)
            v.wait_ge(s_rs, 1)
            for j in range(KT):
                ins = v.tensor_scalar_mul(
                    gg[:, j * NCHK : (j + 1) * NCHK],
                    emat[:, j * NCHK : (j + 1) * NCHK],
                    rS[:, j : j + 1],
                )
            ins.then_inc(s_gq, 1)
            v.wait_ge(s_gq, 1)
            for j in range(KT):
                for c in range(NCHK):
                    idx = j * NCHK + c
                    ins = v.tensor_scalar_mul(
                        vt_all[:, idx * VD : (idx + 1) * VD],
                        v_sb[:, j * VD : (j + 1) * VD],
                        gg[:, idx : idx + 1],
                    )
            ins.then_inc(s_vt, 1)

            # out copies
            for tt in range(8):
                v.wait_ge(s_attn, tt + 1)
                if tt >= 2:
                    v.wait_ge(s_odp[tt % 2], 16 * ((tt - 2) // 2 + 1))
                v.tensor_copy(
                    out_sb[:, (tt % 2) * 1024 : (tt % 2 + 1) * 1024],
                    atile[tt % 4],
                ).then_inc(s_ocp, 1)

    nc.finalize()
    return nc


# ------------------------- host side -------------------------

def _split_bf16(x):
    import ml_dtypes

    hi = x.astype(ml_dtypes.bfloat16)
    lo = (x - hi.astype(np.float32)).astype(ml_dtypes.bfloat16)
    return hi, lo


def _tile_cols(xT, w):
    """[d, w] -> [128, (d//128)*w], col dd*w+i = xT[dd*128+p, i]."""
    dd = xT.shape[0] // 128
    return np.ascontiguousarray(
        xT.reshape(dd, 128, w).transpose(1, 0, 2).reshape(128, dd * w)
    )


def build_inputs(inputs):
    """inputs dict -> (nc, in_maps) for the 8 cores."""
    import ml_dtypes

    bf = ml_dtypes.bfloat16
    queries = np.asarray(inputs["queries"], dtype=np.float32)
    keys = np.asarray(inputs["keys"], dtype=np.float32)
    values = np.asarray(inputs["values"], dtype=np.float32)
    seq, d = queries.shape
    ksl = seq // C

    qw8 = (np.asarray(inputs["query_weights"]) / np.sqrt(np.float32(QK))).astype(
        np.float32
    )
    wqh, wql = _split_bf16(qw8)
    wkh, wkl = _split_bf16(np.asarray(inputs["key_weights"], dtype=np.float32))
    wv = np.asarray(inputs["value_weights"], dtype=np.float32).astype(bf)

    shared = {
        "wqh": _tile_cols(wqh.astype(np.float32), QK).astype(bf),
        "wql": _tile_cols(wql.astype(np.float32), QK).astype(bf),
        "wkh": _tile_cols(wkh.astype(np.float32), QK).astype(bf),
        "wkl": _tile_cols(wkl.astype(np.float32), QK).astype(bf),
        "wv": _tile_cols(wv.astype(np.float32), VD).astype(bf),
    }
    qT = np.ascontiguousarray(queries.T)
    qh, ql = _split_bf16(qT)
    shared["qhT"] = _tile_cols(qh.astype(np.float32), seq).astype(bf)
    shared["qlT"] = _tile_cols(ql.astype(np.float32), seq).astype(bf)

    in_maps = []
    for c in range(C):
        sl = slice(c * ksl, (c + 1) * ksl)
        kT = np.ascontiguousarray(keys[sl].T)
        vTc = np.ascontiguousarray(values[sl].T)
        kh, kl = _split_bf16(kT)
        m = dict(shared)
        m["khT"] = _tile_cols(kh.astype(np.float32), ksl).astype(bf)
        m["klT"] = _tile_cols(kl.astype(np.float32), ksl).astype(bf)
        m["vT"] = _tile_cols(vTc, ksl).astype(bf)
        in_maps.append(m)

    nc = build_nc(seq=seq, d=d)
    return nc, in_maps


def combine_outputs(results):
    acc = np.zeros((VD, SEQ), dtype=np.float32)
    for c in range(C):
        acc += np.asarray(results[c]["out"], dtype=np.float32)
    return np.ascontiguousarray(acc.T)


def run_spmd_staged(nc, in_maps, profile_dir=None):
    """run_bass_via_pjrt with inputs pre-staged on-device (blocks until all
    shards are resident) so the 8 cores launch aligned instead of staggered
    by per-device input-transfer time. Optionally wraps the execute in the
    axon NTFF profile hook (profile_dir)."""
    import jax
    import numpy as np_
    from jax.sharding import Mesh, PartitionSpec, NamedSharding
    from jax.experimental.shard_map import shard_map
    import concourse.mybir as mybir
    from concourse import bass2jax

    bass2jax.install_neuronx_cc_hook()
    n_cores = len(in_maps)

    partition_name = (
        nc.partition_id_tensor.name if nc.partition_id_tensor else None
    )
    in_names, out_names, out_avals, zero_outs = [], [], [], []
    for alloc in nc.m.functions[0].allocations:
        if not isinstance(alloc, mybir.MemoryLocationSet):
            continue
        name = alloc.memorylocations[0].name
        if alloc.kind == "ExternalInput":
            if name != partition_name:
                in_names.append(name)
        elif alloc.kind == "ExternalOutput":
            out_names.append(name)
            shape = tuple(alloc.tensor_shape)
            dtype = mybir.dt.np(alloc.dtype)
            out_avals.append(jax.core.ShapedArray(shape, dtype))
            zero_outs.append(np_.zeros(shape, dtype))
    n_params = len(in_names)
    n_outs = len(out_avals)
    all_names = in_names + out_names
    if partition_name is not None:
        all_names = all_names + [partition_name]

    def _body(*args):
        operands = list(args)
        if partition_name is not None:
            operands.append(bass2jax.partition_id_tensor())
        outs = bass2jax._bass_exec_p.bind(
            *operands,
            out_avals=tuple(out_avals),
            in_names=tuple(all_names),
            out_names=tuple(out_names),
            lowering_input_output_aliases=(),
            sim_require_finite=True,
            sim_require_nnan=True,
            nc=nc,
        )
        return tuple(outs)

    devices = jax.devices()[:n_cores]
    mesh = Mesh(np_.asarray(devices), ("core",))
    spec = NamedSharding(mesh, PartitionSpec("core"))
    sharded = jax.jit(
        shard_map(
            _body,
            mesh=mesh,
            in_specs=(PartitionSpec("core"),) * (n_params + n_outs),
            out_specs=(PartitionSpec("core"),) * n_outs,
            check_rep=False,
        ),
        donate_argnums=tuple(range(n_params, n_params + n_outs)),
        keep_unused=True,
    )
    concat_in = [
        np_.concatenate([np_.asarray(in_maps[c][nm]) for c in range(n_cores)], axis=0)
        for nm in in_names
    ]
    concat_zero = [
        np_.zeros((n_cores * z.shape[0], *z.shape[1:]), z.dtype) for z in zero_outs
    ]
    staged = [jax.device_put(a, spec) for a in concat_in + concat_zero]
    jax.block_until_ready(staged)

    if profile_dir is not None:
        hook = None
        try:
            from antenv.axon_hooks import get_axon_ntff_profile_hook

            hook = get_axon_ntff_profile_hook()
        except ImportError:
            pass
        if hook is None:
            from trn_agent_boot.trn_boot import _ntff_profile_via_ctypes

            hook = _ntff_profile_via_ctypes("/opt/axon/libaxon_pjrt.so")
        with hook(profile_dir, list(range(n_cores))):
            out_arrs = sharded(*staged)
            jax.block_until_ready(out_arrs)
    else:
        out_arrs = sharded(*staged)
    return [
        {
            nm: np_.asarray(out_arrs[i]).reshape(n_cores, *out_avals[i].shape)[c]
            for i, nm in enumerate(out_names)
        }
        for c in range(n_cores)
    ]


def kernel(queries, keys, values, query_weights, key_weights, value_weights):
    import sys

    for p in ("/opt/trn_rl_repo",):
        if p not in sys.path:
            sys.path.insert(0, p)

    nc, in_maps = build_inputs(
        {
            "queries": queries,
            "keys": keys,
            "values": values,
            "query_weights": query_weights,
            "key_weights": key_weights,
            "value_weights": value_weights,
        }
    )
    results = run_spmd_staged(nc, in_maps)
    return combine_outputs(results)
